# revision 20
# baseline (speedup 1.0000x reference)
"""Trainium2 Bass kernel for a GQA attention layer (B=2, S=2048, D=4096,
32 Q heads / 8 KV heads, rotary, additive causal mask), SPMD across 8
NeuronCores.

Sharding: core c = (batch b=c//4, lane l=c%4) owns the four 128-token
query blocks {12+l, 8+l, 4+l, l} of its batch, stored column-major in
that order (descending causal range). The causal structure then becomes
a UNIFORM prefix-width program: key chunk i (of 16) only interacts with
the first w(i) = (4 - i//4)*128 query columns, so scores/exp/AV shrink
to 62.5% of the dense work with the same instruction stream on every
core — all per-core variation lives in input data (token placement,
rotary tables, mask tiles).

K/V projections are computed for local tokens and shared within each
batch's 4 cores via one AllGather (overlapped with the Q projection).
Attention computes transposed scores S^T = K_chunk^T @ Q^T per key
chunk; the additive mask (only ever needed on the last 128 columns of
each prefix) is accumulated INTO PSUM by the PE itself via a
mask-stationary x identity-moving matmul, keeping the critical chain
PE->ACT->PE. exp runs on ACT; the softmax denominator accumulates on
the PE via an all-ones [128,1] stationary into a [2,512] PSUM row pair
(logits are bounded, no max subtraction); normalization is
reciprocal (DVE) -> broadcast matmul (PE) -> multiply (DVE), emitted
deferred into the next head-pair's first chunk to hide latency.

Weights are host-packed so every [128, w] stationary tile is a single
contiguous DMA. The wo projection produces each core's 512 output rows,
scattered back on the host.
"""

import os
import sys
from contextlib import ExitStack
from dataclasses import dataclass

import numpy as np

if os.path.isdir("/opt/trn_rl_repo") and "/opt/trn_rl_repo" not in sys.path:
    sys.path.insert(0, "/opt/trn_rl_repo")

import ml_dtypes

import concourse.bass as bass
import concourse.mybir as mybir
import concourse.tile as tile
from concourse import bacc
from concourse.bass_utils import run_bass_kernel_spmd

BF16 = mybir.dt.bfloat16
F32 = mybir.dt.float32
NPBF16 = ml_dtypes.bfloat16
P = 128


@dataclass(frozen=True)
class Cfg:
    S: int = 2048      # full sequence
    D: int = 4096      # model dim
    NH: int = 32       # query heads
    NKV: int = 8       # kv heads
    HD: int = 128      # head dim (must equal P)

    @property
    def T(self):
        return self.S // 4

    @property
    def TS(self):
        return self.T // P

    @property
    def DT(self):
        return self.D // P

    @property
    def NREP(self):
        return self.NH // self.NKV


FULL = Cfg()

# per-core query blocks for lane l: [12+l, 8+l, 4+l, l] (descending range)
SLOT_R = (16, 12, 8, 4)


def lane_blocks(l):
    return [12 + l, 8 + l, 4 + l, l]


def groups_of3(n):
    return [list(range(k, min(k + 3, n))) for k in range(0, n, 3)]


def pack_colgroups(wT, groups, DT):
    """wT: [D, E] contraction-major. Flat layout: [group][d][128, w_g]
    contiguous blocks."""
    blocks = []
    for grp in groups:
        c0, w = grp[0] * P, len(grp) * P
        for d in range(DT):
            blocks.append(
                np.ascontiguousarray(wT[d * P:(d + 1) * P, c0:c0 + w]).reshape(-1))
    return np.concatenate(blocks)


def build_nc(cfg: Cfg, debug_att=False):
    S, D, NH, NKV, HD = cfg.S, cfg.D, cfg.NH, cfg.NKV, cfg.HD
    T, TS, DT = cfg.T, cfg.TS, cfg.DT
    KVW = NKV * HD
    NCH = 4 * TS                   # 16 global key chunks
    NEH = KVW // 512               # V feature halves
    NDO = D // 512                 # wo output column groups
    SCALE = float(np.float32(1.0) / np.float32(np.sqrt(np.float32(HD))))
    # prefix width per key chunk (query columns ordered by descending range)
    W = [(4 - i // TS) * P for i in range(NCH)]

    kgroups = groups_of3(NKV)
    qgroups = groups_of3(NH)
    vgroups = [(eh, tss) for eh in range(NEH) for tss in groups_of3(TS)]

    nc = bacc.Bacc("TRN2", target_bir_lowering=False, debug=False, num_devices=8)

    xt_d = nc.dram_tensor("xt", [D, T], BF16, kind="ExternalInput")
    wqp_d = nc.dram_tensor("wqp", [D * NH * HD], BF16, kind="ExternalInput")
    wkp_d = nc.dram_tensor("wkp", [D * KVW], BF16, kind="ExternalInput")
    wvp_d = nc.dram_tensor("wvp", [D * KVW], BF16, kind="ExternalInput")
    wop_d = nc.dram_tensor("wop", [NH * HD * D], BF16, kind="ExternalInput")
    cost_d = nc.dram_tensor("cost", [HD, T], F32, kind="ExternalInput")
    sint_d = nc.dram_tensor("sint", [HD, T], F32, kind="ExternalInput")
    # 16 mask tiles [128 q, 128 k] side by side (pre-scaled additive mask)
    maskq_d = nc.dram_tensor("maskq", [P, NCH * P], BF16, kind="ExternalInput")
    ident_d = nc.dram_tensor("ident", [P, 2 * P], BF16, kind="ExternalInput")
    swap_d = nc.dram_tensor("swapm", [P, P], BF16, kind="ExternalInput")
    onesmat_d = nc.dram_tensor("onesmat", [P, P], BF16, kind="ExternalInput")
    out_d = nc.dram_tensor("out", [T, D], F32, kind="ExternalOutput")
    attd_d = (nc.dram_tensor("attd", [NH * HD, T], BF16, kind="ExternalOutput")
              if debug_att else None)

    def grp_offsets(groups):
        offs = []
        off = 0
        for grp in groups:
            offs.append(off)
            off += DT * P * len(grp) * P
        return offs

    qoffs = grp_offsets(qgroups)
    koffs = grp_offsets(kgroups)

    wqp = wqp_d.ap()
    wkp = wkp_d.ap()
    wvp = wvp_d.ap()
    wop = wop_d.ap()

    with tile.TileContext(nc) as tc, ExitStack() as ctx:
        persist = ctx.enter_context(tc.tile_pool(name="persist", bufs=1))
        wpool = ctx.enter_context(tc.tile_pool(name="wpool", bufs=3))
        dramp = ctx.enter_context(tc.tile_pool(name="dramp", bufs=1, space="DRAM"))

        # ---- constants ----
        swap_sb = persist.tile([P, P], BF16, name="swap_sb")
        nc.sync.dma_start(swap_sb[:], swap_d.ap()[:])
        cost_sb = persist.tile([HD, T], F32, name="cost_sb")
        nc.sync.dma_start(cost_sb[:], cost_d.ap()[:])
        sint_sb = persist.tile([HD, T], F32, name="sint_sb")
        nc.sync.dma_start(sint_sb[:], sint_d.ap()[:])
        onesmat_sb = persist.tile([P, P], BF16, name="onesmat_sb")
        nc.sync.dma_start(onesmat_sb[:], onesmat_d.ap()[:])
        ident_sb = persist.tile([P, 2 * P], BF16, name="ident_sb")
        nc.sync.dma_start(ident_sb[:], ident_d.ap()[:])
        maskq_sb = persist.tile([P, NCH * P], BF16, name="maskq_sb")
        nc.sync.dma_start(maskq_sb[:], maskq_d.ap()[:])

        kvin = dramp.tile([2 * KVW, T], BF16, name="kvin")
        kvout = dramp.tile([4 * 2 * KVW, T], BF16, name="kvout")
        kvin_flat = kvin[:].rearrange("a b -> (a b)")
        kvout_flat = kvout[:].rearrange("a b -> (a b)")

        qp = [persist.tile([P, 2 * T], BF16, name=f"qp_{p}") for p in range(NH // 2)]

        with tc.tile_pool(name="xtp", bufs=1) as xtp, \
             tc.tile_pool(name="rot", bufs=2) as rot, \
             tc.tile_pool(name="psP", bufs=1, space="PSUM") as psP:

            def rotary(raw_ps, dst_ap, nm):
                """Interleaved rotary on a [P, T] feature-transposed PSUM tile."""
                raw = rot.tile([P, T], BF16, tag="raw", bufs=6, name=f"raw_{nm}")
                nc.scalar.copy(raw[:], raw_ps[:])
                sw_ps = psP.tile([P, T], F32, tag="swp", bufs=2, name=f"swp_{nm}")
                nc.tensor.matmul(sw_ps[:], swap_sb[:], raw[:], start=True, stop=True)
                t1 = rot.tile([P, T], F32, tag="t1", bufs=4, name=f"t1_{nm}")
                nc.vector.tensor_mul(t1[:], raw[:], cost_sb[:])
                t2 = rot.tile([P, T], F32, tag="t2", bufs=4, name=f"t2_{nm}")
                nc.vector.tensor_mul(t2[:], sw_ps[:], sint_sb[:])
                nc.vector.tensor_add(dst_ap, t1[:], t2[:])

            xt_sb = [xtp.tile([P, T], BF16, name=f"xt_sb_{d}") for d in range(DT)]
            xt_loaded = [False] * DT

            def load_xt(d):
                if not xt_loaded[d]:
                    nc.sync.dma_start(xt_sb[d][:],
                                      xt_d.ap()[d * P:(d + 1) * P, :])
                    xt_loaded[d] = True

            # ---- K^T projection (local tokens) + rotary ----
            ktloc = [xtp.tile([P, T], BF16, name=f"ktloc_{kvh}")
                     for kvh in range(NKV)]
            for gi, grp in enumerate(kgroups):
                w = len(grp) * P
                kps = [psP.tile([P, T], F32, tag=f"pj{j}", bufs=2,
                                name=f"kps_{gi}_{j}") for j in range(len(grp))]
                for d in range(DT):
                    wrow = wpool.tile([P, 3 * P], BF16, tag="wkv", bufs=18,
                                      name=f"wk_{gi}_{d}")
                    off = koffs[gi] + d * P * w
                    nc.sync.dma_start(
                        wrow[:, :w],
                        wkp[off:off + P * w].rearrange("(p f) -> p f", p=P))
                    load_xt(d)
                    for j in range(len(grp)):
                        nc.tensor.matmul(
                            kps[j][:], wrow[:, j * HD:(j + 1) * HD], xt_sb[d][:],
                            start=(d == 0), stop=(d == DT - 1))
                for j, kvh in enumerate(grp):
                    rotary(kps[j], ktloc[kvh][:], f"k{kvh}")

            # ---- V projection (local tokens), [token, feature] layout ----
            vtloc = [xtp.tile([P, KVW], BF16, name=f"vtloc_{ts}")
                     for ts in range(TS)]
            for gi, (eh, tss) in enumerate(vgroups):
                vps = [psP.tile([P, 512], F32, tag=f"pj{j}", bufs=2,
                                name=f"vps_{gi}_{j}") for j in range(len(tss))]
                for d in range(DT):
                    wrow = wpool.tile([P, 512], BF16, tag="wvr", bufs=9,
                                      name=f"wv_{gi}_{d}")
                    off = (eh * DT + d) * P * 512
                    nc.sync.dma_start(
                        wrow[:],
                        wvp[off:off + P * 512].rearrange("(p f) -> p f", p=P))
                    for j, ts in enumerate(tss):
                        nc.tensor.matmul(
                            vps[j][:], xt_sb[d][:, ts * P:(ts + 1) * P], wrow[:],
                            start=(d == 0), stop=(d == DT - 1))
                for j, ts in enumerate(tss):
                    nc.scalar.copy(vtloc[ts][:, eh * 512:(eh + 1) * 512], vps[j][:])

            # ---- pack K^T and V into the collective input buffer ----
            for kvh in range(NKV):
                nc.sync.dma_start(kvin[kvh * HD:(kvh + 1) * HD, :], ktloc[kvh][:])
            vbase = KVW * T
            for kvh in range(NKV):
                for ts in range(TS):
                    off = vbase + (kvh * TS + ts) * P * HD
                    nc.sync.dma_start(
                        kvin_flat[off:off + P * HD]
                        .rearrange("(p f) -> p f", p=P),
                        vtloc[ts][:, kvh * HD:(kvh + 1) * HD])

            nc.gpsimd.collective_compute(
                "AllGather",
                mybir.AluOpType.bypass,
                replica_groups=[[0, 1, 2, 3], [4, 5, 6, 7]],
                ins=[kvin[:].opt()],
                outs=[kvout[:].opt()],
            )

            # ---- Q^T projection + rotary (overlaps the AllGather) ----
            for gi, grp in enumerate(qgroups):
                w = len(grp) * P
                qps = [psP.tile([P, T], F32, tag=f"pj{j}", bufs=2,
                                name=f"qps_{gi}_{j}") for j in range(len(grp))]
                for d in range(DT):
                    wrow = wpool.tile([P, 3 * P], BF16, tag="wq", bufs=32,
                                      name=f"wq_{gi}_{d}")
                    off = qoffs[gi] + d * P * w
                    nc.sync.dma_start(
                        wrow[:, :w],
                        wqp[off:off + P * w].rearrange("(p f) -> p f", p=P))
                    for j in range(len(grp)):
                        nc.tensor.matmul(
                            qps[j][:], wrow[:, j * HD:(j + 1) * HD], xt_sb[d][:],
                            start=(d == 0), stop=(d == DT - 1))
                for j, h in enumerate(grp):
                    rotary(qps[j], qp[h // 2][:, (h % 2) * T:(h % 2 + 1) * T],
                           f"q{h}")

        tc.no_sync_barrier()

        # ---- attention: prefix-width causal sweep per (kv head, head pair) ----
        att = [persist.tile([P, T], BF16, name=f"att_{h}") for h in range(NH)]
        EXPF = mybir.ActivationFunctionType.Exp
        with tc.tile_pool(name="kvp", bufs=1) as kvp, \
             tc.tile_pool(name="atw", bufs=1) as work, \
             tc.tile_pool(name="psA", bufs=1, space="PSUM") as psA:

            pending = [None]

            def flush_pending():
                av_p, zacc_p, hs_p = pending[0]
                for j, h in enumerate(hs_p):
                    zbb = psA.tile([P, 2 * T], F32, tag="s", bufs=2,
                                   name=f"zbb_{h}")
                    nc.tensor.matmul(zbb[:, :T], onesmat_sb[:],
                                     zacc_p[:, j * T:(j + 1) * T],
                                     start=True, stop=True)
                    rz = work.tile([P, T], F32, tag="rz", bufs=2,
                                   name=f"rz_{h}")
                    nc.vector.reciprocal_approx_fast(out=rz[:], in_=zbb[:, :T])
                    nc.vector.tensor_mul(att[h][:], av_p[:, j * T:(j + 1) * T], rz[:])
                pending[0] = None

            for kvh in range(NKV):
                # gathered K^T [hd, tok] and V [tok, hd] tiles per source lane
                ktl = []
                vtl = []
                for r in range(4):
                    kt_t = kvp.tile([P, T], BF16, tag="kt", bufs=8,
                                    name=f"kt_{kvh}_{r}")
                    nc.sync.dma_start(
                        kt_t[:],
                        kvout[r * 2 * KVW + kvh * HD: r * 2 * KVW + (kvh + 1) * HD, :])
                    ktl.append(kt_t)
                    vt_t = kvp.tile([P, T], BF16, tag="vt", bufs=8,
                                    name=f"vt_{kvh}_{r}")
                    for ts in range(TS):
                        off = (r * 2 * KVW + KVW) * T + (kvh * TS + ts) * P * HD
                        nc.sync.dma_start(
                            vt_t[:, ts * P:(ts + 1) * P],
                            kvout_flat[off:off + P * HD]
                            .rearrange("(p f) -> p f", p=P))
                    vtl.append(vt_t)

                for sub in range(cfg.NREP // 2):
                    h0 = kvh * cfg.NREP + sub * 2
                    hs = (h0, h0 + 1)
                    av = psA.tile([P, 2 * T], F32, tag="av", bufs=2,
                                  name=f"av_{h0}")
                    av3 = av[:].rearrange("p (g t) -> p g t", g=2)
                    zacc = work.tile([P, 2 * T], BF16, tag="zacc", bufs=2,
                                     name=f"zacc_{h0}")
                    zacc3 = zacc[:].rearrange("p (g t) -> p g t", g=2)
                    qpt = qp[h0 // 2]
                    for i in range(NCH):
                        w = W[i]
                        lane, pos = i % 4, TS - 1 - i // TS
                        ksl = ktl[lane][:, pos * P:(pos + 1) * P]
                        vsl = vtl[lane][:, pos * P:(pos + 1) * P]
                        s = psA.tile([P, 2 * T], F32, tag="s", bufs=2,
                                     name=f"s_{h0}_{i}")
                        s3 = s[:].rearrange("p (g t) -> p g t", g=2)
                        for j in range(2):
                            nc.tensor.matmul(
                                s[:, j * T:j * T + w], ksl,
                                qpt[:, j * T:j * T + w],
                                start=True, stop=False, skip_group_check=True)
                        nc.tensor.matmul(
                            s3[:, :, w - P:w],
                            maskq_sb[:, i * P:(i + 1) * P], ident_sb[:],
                            start=False, stop=True, skip_group_check=True)
                        if i == 0 and pending[0] is not None:
                            flush_pending()
                        e = work.tile([P, 2 * T], BF16, tag="e", bufs=4,
                                      name=f"e_{h0}_{i}")
                        e3 = e[:].rearrange("p (g t) -> p g t", g=2)
                        nc.scalar.activation(e3[:, :, :w], s3[:, :, :w],
                                             EXPF, scale=SCALE)
                        for j in range(2):
                            nc.tensor.matmul(
                                av[:, j * T:j * T + w], vsl,
                                e[:, j * T:j * T + w],
                                start=(i == 0), stop=(i == NCH - 1),
                                skip_group_check=True)
                        if i == 0:
                            nc.vector.tensor_scalar_add(
                                zacc3[:, :, :w], e3[:, :, :w], 0.0)
                        else:
                            nc.vector.tensor_add(
                                zacc3[:, :, :w], zacc3[:, :, :w],
                                e3[:, :, :w])
                    pending[0] = (av, zacc, hs)
            flush_pending()
            if debug_att:
                for h in range(NH):
                    nc.sync.dma_start(
                        attd_d.ap()[h * HD:(h + 1) * HD, :], att[h][:])

        tc.no_sync_barrier()

        # ---- output projection ----
        with tc.tile_pool(name="osbp", bufs=1) as osbp, \
             tc.tile_pool(name="psW", bufs=1, space="PSUM") as psW:
            for douth in range(NDO):
                ops = [psW.tile([P, 512], F32, tag=f"pw{tt}", bufs=2,
                                name=f"ops_{douth}_{tt}") for tt in range(TS)]
                for e in range(NH):
                    wrow = wpool.tile([P, 512], BF16, tag="wo", bufs=16,
                                      name=f"wo_{douth}_{e}")
                    off = (douth * NH + e) * P * 512
                    nc.sync.dma_start(
                        wrow[:],
                        wop[off:off + P * 512].rearrange("(p f) -> p f", p=P))
                    for tt in range(TS):
                        nc.tensor.matmul(
                            ops[tt][:], att[e][:, tt * P:(tt + 1) * P], wrow[:],
                            start=(e == 0), stop=(e == NH - 1))
                for tt in range(TS):
                    osb = osbp.tile([P, 512], F32, tag="osb", bufs=4,
                                    name=f"osb_{douth}_{tt}")
                    nc.scalar.copy(osb[:], ops[tt][:])
                    nc.sync.dma_start(
                        out_d.ap()[tt * P:(tt + 1) * P, douth * 512:(douth + 1) * 512],
                        osb[:])

    nc.compile()
    return nc


def check_mask_structure(mask, cfg: Cfg):
    """The program computes, for lane l's slot-R block, key chunks [0, R).
    Correctness requires the mask to be fully-closed beyond R and fully-open
    below R-4 (the windowed region is handled by data tiles)."""
    for l in range(4):
        for si, b in enumerate(lane_blocks(l)):
            R = SLOT_R[si]
            qs = slice(b * P, (b + 1) * P)
            if R * P < cfg.S:
                if not (mask[qs, R * P:] <= -1e8).all():
                    return False
            lo = max(0, (R - 4)) * P
            if not (mask[qs, :lo] == 0).all():
                return False
    return True


def make_in_maps(x, freqs_cis, mask, wq, wk, wv, wo, cfg: Cfg):
    S, D, T, HD, DT = cfg.S, cfg.D, cfg.T, cfg.HD, cfg.DT
    NCH = 4 * cfg.TS
    NEH = cfg.NKV * HD // 512
    NDO = D // 512
    SCALE = np.float32(1.0) / np.float32(np.sqrt(np.float32(HD)))
    x = np.asarray(x, np.float32)
    fc = np.asarray(freqs_cis, np.float32)
    mask = np.asarray(mask, np.float32)
    wqt = np.asarray(wq, np.float32).T.astype(NPBF16)   # [D, NH*HD]
    wkt = np.asarray(wk, np.float32).T.astype(NPBF16)   # [D, KVW]
    wvt = np.asarray(wv, np.float32).T.astype(NPBF16)
    wot = np.asarray(wo, np.float32).T.astype(NPBF16)   # [NH*HD, D]

    wqp = pack_colgroups(wqt, groups_of3(cfg.NH), DT)
    wkp = pack_colgroups(wkt, groups_of3(cfg.NKV), DT)
    wvp = np.concatenate([
        np.ascontiguousarray(wvt[d * P:(d + 1) * P, eh * 512:(eh + 1) * 512])
        .reshape(-1)
        for eh in range(NEH) for d in range(DT)])
    wop = np.concatenate([
        np.ascontiguousarray(wot[e * P:(e + 1) * P, douth * 512:(douth + 1) * 512])
        .reshape(-1)
        for douth in range(NDO) for e in range(cfg.NH)])

    swapm = np.zeros((P, P), np.float32)
    for i in range(P // 2):
        swapm[2 * i, 2 * i + 1] = 1.0
        swapm[2 * i + 1, 2 * i] = 1.0
    swapm = swapm.astype(NPBF16)
    onesmat = np.ones((P, P), NPBF16)
    ident = np.concatenate([np.eye(P, dtype=np.float32)] * 2, axis=1).astype(NPBF16)

    in_maps = []
    for c in range(8):
        b, l = c // 4, c % 4
        blocks = lane_blocks(l)
        toks = np.concatenate([np.arange(bb * P, (bb + 1) * P) for bb in blocks])
        xt = np.ascontiguousarray(x[b][toks, :].T).astype(NPBF16)
        cost = np.repeat(fc[toks, :, 0].T, 2, axis=0).astype(np.float32)
        sint = np.repeat(fc[toks, :, 1].T, 2, axis=0).astype(np.float32)
        sint[0::2, :] *= -1.0
        # mask tile for chunk i: [128 q of the masked slot block, 128 k]
        maskq = np.zeros((P, NCH * P), np.float32)
        for i in range(NCH):
            si = 3 - i // 4                     # slot index masked by chunk i
            bb = blocks[si]
            maskq[:, i * P:(i + 1) * P] = \
                mask[bb * P:(bb + 1) * P, i * P:(i + 1) * P] / SCALE
        in_maps.append({
            "xt": xt, "wqp": wqp, "wkp": wkp, "wvp": wvp, "wop": wop,
            "cost": np.ascontiguousarray(cost),
            "sint": np.ascontiguousarray(sint),
            "maskq": np.ascontiguousarray(maskq).astype(NPBF16),
            "ident": ident, "swapm": swapm, "onesmat": onesmat,
        })
    return in_maps


_NC_CACHE = {}


def kernel_run(x, start_pos, freqs_cis, mask, wq, wk, wv, wo,
               cfg: Cfg = FULL, trace=False):
    mask_np = np.asarray(mask, np.float32)
    assert check_mask_structure(mask_np, cfg), \
        "mask incompatible with compiled causal structure"
    in_maps = make_in_maps(x, freqs_cis, mask, wq, wk, wv, wo, cfg)
    if cfg not in _NC_CACHE:
        _NC_CACHE[cfg] = build_nc(cfg)
    nc = _NC_CACHE[cfg]
    res = run_bass_kernel_spmd(nc, in_maps, core_ids=list(range(8)), trace=trace)
    full = np.zeros((2, cfg.S, cfg.D), np.float32)
    for c in range(8):
        b, l = c // 4, c % 4
        toks = np.concatenate(
            [np.arange(bb * P, (bb + 1) * P) for bb in lane_blocks(l)])
        full[b][toks, :] = res.results[c]["out"]
    return full, res


def kernel(x, start_pos=None, freqs_cis=None, mask=None, wq=None, wk=None,
           wv=None, wo=None):
    full, _ = kernel_run(x, start_pos, freqs_cis, mask, wq, wk, wv, wo)
    return full


# revision 21
# speedup vs baseline: 1.0169x; 1.0169x over previous
"""Trainium2 Bass kernel for a GQA attention layer (B=2, S=2048, D=4096,
32 Q heads / 8 KV heads, rotary, additive causal mask), SPMD across 8
NeuronCores.

Sharding: core c = (batch b=c//4, lane l=c%4) owns the four 128-token
query blocks {12+l, 8+l, 4+l, l} of its batch, stored column-major in
that order (descending causal range). The causal structure then becomes
a UNIFORM prefix-width program: key chunk i (of 16) only interacts with
the first w(i) = (4 - i//4)*128 query columns, so scores/exp/AV shrink
to 62.5% of the dense work with the same instruction stream on every
core — all per-core variation lives in input data (token placement,
rotary tables, mask tiles).

K/V projections are computed for local tokens and shared within each
batch's 4 cores via one AllGather (overlapped with the Q projection).
Attention computes transposed scores S^T = K_chunk^T @ Q^T per key
chunk; the additive mask (only ever needed on the last 128 columns of
each prefix) is accumulated INTO PSUM by the PE itself via a
mask-stationary x identity-moving matmul, keeping the critical chain
PE->ACT->PE. exp runs on ACT; the softmax denominator accumulates on
the PE via an all-ones [128,1] stationary into a [2,512] PSUM row pair
(logits are bounded, no max subtraction); normalization is
reciprocal (DVE) -> broadcast matmul (PE) -> multiply (DVE), emitted
deferred into the next head-pair's first chunk to hide latency.

Weights are host-packed so every [128, w] stationary tile is a single
contiguous DMA. The wo projection produces each core's 512 output rows,
scattered back on the host.
"""

import os
import sys
from contextlib import ExitStack
from dataclasses import dataclass

import numpy as np

if os.path.isdir("/opt/trn_rl_repo") and "/opt/trn_rl_repo" not in sys.path:
    sys.path.insert(0, "/opt/trn_rl_repo")

import ml_dtypes

import concourse.bass as bass
import concourse.mybir as mybir
import concourse.tile as tile
from concourse import bacc
from concourse.bass_utils import run_bass_kernel_spmd

BF16 = mybir.dt.bfloat16
F32 = mybir.dt.float32
NPBF16 = ml_dtypes.bfloat16
P = 128


@dataclass(frozen=True)
class Cfg:
    S: int = 2048      # full sequence
    D: int = 4096      # model dim
    NH: int = 32       # query heads
    NKV: int = 8       # kv heads
    HD: int = 128      # head dim (must equal P)

    @property
    def T(self):
        return self.S // 4

    @property
    def TS(self):
        return self.T // P

    @property
    def DT(self):
        return self.D // P

    @property
    def NREP(self):
        return self.NH // self.NKV


FULL = Cfg()

# per-core query blocks for lane l: [12+l, 8+l, 4+l, l] (descending range)
SLOT_R = (16, 12, 8, 4)


def lane_blocks(l):
    return [12 + l, 8 + l, 4 + l, l]


def groups_of3(n):
    return [list(range(k, min(k + 3, n))) for k in range(0, n, 3)]


def pack_colgroups(wT, groups, DT):
    """wT: [D, E] contraction-major. Flat layout: [group][d][128, w_g]
    contiguous blocks."""
    blocks = []
    for grp in groups:
        c0, w = grp[0] * P, len(grp) * P
        for d in range(DT):
            blocks.append(
                np.ascontiguousarray(wT[d * P:(d + 1) * P, c0:c0 + w]).reshape(-1))
    return np.concatenate(blocks)


def build_nc(cfg: Cfg, debug_att=False):
    S, D, NH, NKV, HD = cfg.S, cfg.D, cfg.NH, cfg.NKV, cfg.HD
    T, TS, DT = cfg.T, cfg.TS, cfg.DT
    KVW = NKV * HD
    NCH = 4 * TS                   # 16 global key chunks
    NEH = KVW // 512               # V feature halves
    NDO = D // 512                 # wo output column groups
    SCALE = float(np.float32(1.0) / np.float32(np.sqrt(np.float32(HD))))
    # prefix width per key chunk (query columns ordered by descending range)
    W = [(4 - i // TS) * P for i in range(NCH)]

    kgroups = groups_of3(NKV)
    qgroups = groups_of3(NH)
    vgroups = [(eh, tss) for eh in range(NEH) for tss in groups_of3(TS)]

    nc = bacc.Bacc("TRN2", target_bir_lowering=False, debug=False, num_devices=8)

    xt_d = nc.dram_tensor("xt", [D, T], BF16, kind="ExternalInput")
    wqp_d = nc.dram_tensor("wqp", [D * NH * HD], BF16, kind="ExternalInput")
    wkp_d = nc.dram_tensor("wkp", [D * KVW], BF16, kind="ExternalInput")
    wvp_d = nc.dram_tensor("wvp", [D * KVW], BF16, kind="ExternalInput")
    wop_d = nc.dram_tensor("wop", [NH * HD * D], BF16, kind="ExternalInput")
    cost_d = nc.dram_tensor("cost", [HD, T], F32, kind="ExternalInput")
    sint_d = nc.dram_tensor("sint", [HD, T], F32, kind="ExternalInput")
    # 16 mask tiles [128 q, 128 k] side by side (pre-scaled additive mask)
    maskq_d = nc.dram_tensor("maskq", [P, NCH * P], BF16, kind="ExternalInput")
    ident_d = nc.dram_tensor("ident", [P, 2 * P], BF16, kind="ExternalInput")
    swap_d = nc.dram_tensor("swapm", [P, P], BF16, kind="ExternalInput")
    onesmat_d = nc.dram_tensor("onesmat", [P, P], BF16, kind="ExternalInput")
    out_d = nc.dram_tensor("out", [T, D], F32, kind="ExternalOutput")
    attd_d = (nc.dram_tensor("attd", [NH * HD, T], BF16, kind="ExternalOutput")
              if debug_att else None)

    def grp_offsets(groups):
        offs = []
        off = 0
        for grp in groups:
            offs.append(off)
            off += DT * P * len(grp) * P
        return offs

    qoffs = grp_offsets(qgroups)
    koffs = grp_offsets(kgroups)

    wqp = wqp_d.ap()
    wkp = wkp_d.ap()
    wvp = wvp_d.ap()
    wop = wop_d.ap()

    with tile.TileContext(nc) as tc, ExitStack() as ctx:
        persist = ctx.enter_context(tc.tile_pool(name="persist", bufs=1))
        wpool = ctx.enter_context(tc.tile_pool(name="wpool", bufs=3))
        dramp = ctx.enter_context(tc.tile_pool(name="dramp", bufs=1, space="DRAM"))

        # ---- constants ----
        swap_sb = persist.tile([P, P], BF16, name="swap_sb")
        nc.sync.dma_start(swap_sb[:], swap_d.ap()[:])
        cost_sb = persist.tile([HD, T], F32, name="cost_sb")
        nc.sync.dma_start(cost_sb[:], cost_d.ap()[:])
        sint_sb = persist.tile([HD, T], F32, name="sint_sb")
        nc.sync.dma_start(sint_sb[:], sint_d.ap()[:])
        onesmat_sb = persist.tile([P, P], BF16, name="onesmat_sb")
        nc.sync.dma_start(onesmat_sb[:], onesmat_d.ap()[:])
        ident_sb = persist.tile([P, 2 * P], BF16, name="ident_sb")
        nc.sync.dma_start(ident_sb[:], ident_d.ap()[:])
        maskq_sb = persist.tile([P, NCH * P], BF16, name="maskq_sb")
        nc.sync.dma_start(maskq_sb[:], maskq_d.ap()[:])

        kvin = dramp.tile([2 * KVW, T], BF16, name="kvin")
        kvout = dramp.tile([4 * 2 * KVW, T], BF16, name="kvout")
        kvin_flat = kvin[:].rearrange("a b -> (a b)")
        kvout_flat = kvout[:].rearrange("a b -> (a b)")

        qp = [persist.tile([P, 2 * T], BF16, name=f"qp_{p}") for p in range(NH // 2)]

        with tc.tile_pool(name="xtp", bufs=1) as xtp, \
             tc.tile_pool(name="rot", bufs=2) as rot, \
             tc.tile_pool(name="psP", bufs=1, space="PSUM") as psP:

            def rotary(raw_ps, dst_ap, nm):
                """Interleaved rotary on a [P, T] feature-transposed PSUM tile."""
                raw = rot.tile([P, T], BF16, tag="raw", bufs=6, name=f"raw_{nm}")
                nc.scalar.copy(raw[:], raw_ps[:])
                sw_ps = psP.tile([P, T], F32, tag="swp", bufs=2, name=f"swp_{nm}")
                nc.tensor.matmul(sw_ps[:], swap_sb[:], raw[:], start=True, stop=True)
                t1 = rot.tile([P, T], F32, tag="t1", bufs=4, name=f"t1_{nm}")
                nc.vector.tensor_mul(t1[:], raw[:], cost_sb[:])
                t2 = rot.tile([P, T], F32, tag="t2", bufs=4, name=f"t2_{nm}")
                nc.vector.tensor_mul(t2[:], sw_ps[:], sint_sb[:])
                nc.vector.tensor_add(dst_ap, t1[:], t2[:])

            xt_sb = [xtp.tile([P, T], BF16, name=f"xt_sb_{d}") for d in range(DT)]
            xt_loaded = [False] * DT

            def load_xt(d):
                if not xt_loaded[d]:
                    nc.sync.dma_start(xt_sb[d][:],
                                      xt_d.ap()[d * P:(d + 1) * P, :])
                    xt_loaded[d] = True

            # ---- K^T projection (local tokens) + rotary ----
            ktloc = [xtp.tile([P, T], BF16, name=f"ktloc_{kvh}")
                     for kvh in range(NKV)]
            for gi, grp in enumerate(kgroups):
                w = len(grp) * P
                kps = [psP.tile([P, T], F32, tag=f"pj{j}", bufs=2,
                                name=f"kps_{gi}_{j}") for j in range(len(grp))]
                for d in range(DT):
                    wrow = wpool.tile([P, 3 * P], BF16, tag="wkv", bufs=16,
                                      name=f"wk_{gi}_{d}")
                    off = koffs[gi] + d * P * w
                    nc.sync.dma_start(
                        wrow[:, :w],
                        wkp[off:off + P * w].rearrange("(p f) -> p f", p=P))
                    load_xt(d)
                    for j in range(len(grp)):
                        nc.tensor.matmul(
                            kps[j][:], wrow[:, j * HD:(j + 1) * HD], xt_sb[d][:],
                            start=(d == 0), stop=(d == DT - 1))
                for j, kvh in enumerate(grp):
                    rotary(kps[j], ktloc[kvh][:], f"k{kvh}")

            # ---- V projection (local tokens), [token, feature] layout ----
            vtloc = [xtp.tile([P, KVW], BF16, name=f"vtloc_{ts}")
                     for ts in range(TS)]
            for gi, (eh, tss) in enumerate(vgroups):
                vps = [psP.tile([P, 512], F32, tag=f"pj{j}", bufs=2,
                                name=f"vps_{gi}_{j}") for j in range(len(tss))]
                for d in range(DT):
                    wrow = wpool.tile([P, 512], BF16, tag="wvr", bufs=8,
                                      name=f"wv_{gi}_{d}")
                    off = (eh * DT + d) * P * 512
                    nc.sync.dma_start(
                        wrow[:],
                        wvp[off:off + P * 512].rearrange("(p f) -> p f", p=P))
                    for j, ts in enumerate(tss):
                        nc.tensor.matmul(
                            vps[j][:], xt_sb[d][:, ts * P:(ts + 1) * P], wrow[:],
                            start=(d == 0), stop=(d == DT - 1))
                for j, ts in enumerate(tss):
                    nc.scalar.copy(vtloc[ts][:, eh * 512:(eh + 1) * 512], vps[j][:])

            # ---- pack K^T and V into the collective input buffer ----
            for kvh in range(NKV):
                nc.sync.dma_start(kvin[kvh * HD:(kvh + 1) * HD, :], ktloc[kvh][:])
            vbase = KVW * T
            for kvh in range(NKV):
                for ts in range(TS):
                    off = vbase + (kvh * TS + ts) * P * HD
                    nc.sync.dma_start(
                        kvin_flat[off:off + P * HD]
                        .rearrange("(p f) -> p f", p=P),
                        vtloc[ts][:, kvh * HD:(kvh + 1) * HD])

            nc.gpsimd.collective_compute(
                "AllGather",
                mybir.AluOpType.bypass,
                replica_groups=[[0, 1, 2, 3], [4, 5, 6, 7]],
                ins=[kvin[:].opt()],
                outs=[kvout[:].opt()],
            )

            # ---- Q^T projection + rotary (overlaps the AllGather) ----
            for gi, grp in enumerate(qgroups):
                w = len(grp) * P
                qps = [psP.tile([P, T], F32, tag=f"pj{j}", bufs=2,
                                name=f"qps_{gi}_{j}") for j in range(len(grp))]
                for d in range(DT):
                    wrow = wpool.tile([P, 3 * P], BF16, tag="wq", bufs=32,
                                      name=f"wq_{gi}_{d}")
                    off = qoffs[gi] + d * P * w
                    nc.sync.dma_start(
                        wrow[:, :w],
                        wqp[off:off + P * w].rearrange("(p f) -> p f", p=P))
                    for j in range(len(grp)):
                        nc.tensor.matmul(
                            qps[j][:], wrow[:, j * HD:(j + 1) * HD], xt_sb[d][:],
                            start=(d == 0), stop=(d == DT - 1))
                for j, h in enumerate(grp):
                    rotary(qps[j], qp[h // 2][:, (h % 2) * T:(h % 2 + 1) * T],
                           f"q{h}")

        tc.no_sync_barrier()

        # ---- attention: prefix-width causal sweep per (kv head, head pair) ----
        att = [persist.tile([P, T], BF16, name=f"att_{h}") for h in range(NH)]
        EXPF = mybir.ActivationFunctionType.Exp
        with tc.tile_pool(name="kvp", bufs=1) as kvp, \
             tc.tile_pool(name="atw", bufs=1) as work, \
             tc.tile_pool(name="psA", bufs=1, space="PSUM") as psA:

            pending = [None]

            def flush_pending():
                av_p, zacc_p, hs_p = pending[0]
                for j, h in enumerate(hs_p):
                    zbb = psA.tile([P, 2 * T], F32, tag="s", bufs=2,
                                   name=f"zbb_{h}")
                    nc.tensor.matmul(zbb[:, :T], onesmat_sb[:],
                                     zacc_p[:, j * T:(j + 1) * T],
                                     start=True, stop=True)
                    rz = work.tile([P, T], F32, tag="rz", bufs=2,
                                   name=f"rz_{h}")
                    nc.vector.reciprocal_approx_fast(out=rz[:], in_=zbb[:, :T])
                    nc.vector.tensor_mul(att[h][:], av_p[:, j * T:(j + 1) * T], rz[:])
                pending[0] = None

            for kvh in range(NKV):
                # gathered K^T [hd, tok] and V [tok, hd] tiles per source lane
                ktl = []
                vtl = []
                for r in range(4):
                    kt_t = kvp.tile([P, T], BF16, tag="kt", bufs=8,
                                    name=f"kt_{kvh}_{r}")
                    nc.sync.dma_start(
                        kt_t[:],
                        kvout[r * 2 * KVW + kvh * HD: r * 2 * KVW + (kvh + 1) * HD, :])
                    ktl.append(kt_t)
                    vt_t = kvp.tile([P, T], BF16, tag="vt", bufs=8,
                                    name=f"vt_{kvh}_{r}")
                    for ts in range(TS):
                        off = (r * 2 * KVW + KVW) * T + (kvh * TS + ts) * P * HD
                        nc.sync.dma_start(
                            vt_t[:, ts * P:(ts + 1) * P],
                            kvout_flat[off:off + P * HD]
                            .rearrange("(p f) -> p f", p=P))
                    vtl.append(vt_t)

                for sub in range(cfg.NREP // 2):
                    h0 = kvh * cfg.NREP + sub * 2
                    hs = (h0, h0 + 1)
                    av = psA.tile([P, 2 * T], F32, tag="av", bufs=2,
                                  name=f"av_{h0}")
                    av3 = av[:].rearrange("p (g t) -> p g t", g=2)
                    zacc = work.tile([P, 2 * T], BF16, tag="zacc", bufs=2,
                                     name=f"zacc_{h0}")
                    zacc3 = zacc[:].rearrange("p (g t) -> p g t", g=2)
                    qpt = qp[h0 // 2]
                    for i in range(NCH):
                        w = W[i]
                        lane, pos = i % 4, TS - 1 - i // TS
                        ksl = ktl[lane][:, pos * P:(pos + 1) * P]
                        vsl = vtl[lane][:, pos * P:(pos + 1) * P]
                        s = psA.tile([P, 2 * T], F32, tag="s", bufs=2,
                                     name=f"s_{h0}_{i}")
                        s3 = s[:].rearrange("p (g t) -> p g t", g=2)
                        for j in range(2):
                            nc.tensor.matmul(
                                s[:, j * T:j * T + w], ksl,
                                qpt[:, j * T:j * T + w],
                                start=True, stop=False, skip_group_check=True)
                        nc.tensor.matmul(
                            s3[:, :, w - P:w],
                            maskq_sb[:, i * P:(i + 1) * P], ident_sb[:],
                            start=False, stop=True, skip_group_check=True)
                        if i == 0 and pending[0] is not None:
                            flush_pending()
                        e = work.tile([P, 2 * T], BF16, tag="e", bufs=4,
                                      name=f"e_{h0}_{i}")
                        e3 = e[:].rearrange("p (g t) -> p g t", g=2)
                        nc.scalar.activation(e3[:, :, :w], s3[:, :, :w],
                                             EXPF, scale=SCALE)
                        for j in range(2):
                            nc.tensor.matmul(
                                av[:, j * T:j * T + w], vsl,
                                e[:, j * T:j * T + w],
                                start=(i == 0), stop=(i == NCH - 1),
                                skip_group_check=True)
                        if i == 0:
                            nc.vector.tensor_scalar_add(
                                zacc3[:, :, :w], e3[:, :, :w], 0.0)
                        else:
                            nc.vector.tensor_add(
                                zacc3[:, :, :w], zacc3[:, :, :w],
                                e3[:, :, :w])
                    pending[0] = (av, zacc, hs)
            flush_pending()
            if debug_att:
                for h in range(NH):
                    nc.sync.dma_start(
                        attd_d.ap()[h * HD:(h + 1) * HD, :], att[h][:])

        tc.no_sync_barrier()

        # ---- output projection ----
        with tc.tile_pool(name="osbp", bufs=1) as osbp, \
             tc.tile_pool(name="psW", bufs=1, space="PSUM") as psW:
            for douth in range(NDO):
                ops = [psW.tile([P, 512], F32, tag=f"pw{tt}", bufs=2,
                                name=f"ops_{douth}_{tt}") for tt in range(TS)]
                for e in range(NH):
                    wrow = wpool.tile([P, 512], BF16, tag="wo", bufs=16,
                                      name=f"wo_{douth}_{e}")
                    off = (douth * NH + e) * P * 512
                    nc.sync.dma_start(
                        wrow[:],
                        wop[off:off + P * 512].rearrange("(p f) -> p f", p=P))
                    for tt in range(TS):
                        nc.tensor.matmul(
                            ops[tt][:], att[e][:, tt * P:(tt + 1) * P], wrow[:],
                            start=(e == 0), stop=(e == NH - 1))
                for tt in range(TS):
                    osb = osbp.tile([P, 512], F32, tag="osb", bufs=4,
                                    name=f"osb_{douth}_{tt}")
                    nc.scalar.copy(osb[:], ops[tt][:])
                    nc.sync.dma_start(
                        out_d.ap()[tt * P:(tt + 1) * P, douth * 512:(douth + 1) * 512],
                        osb[:])

    nc.compile()
    return nc


def check_mask_structure(mask, cfg: Cfg):
    """The program computes, for lane l's slot-R block, key chunks [0, R).
    Correctness requires the mask to be fully-closed beyond R and fully-open
    below R-4 (the windowed region is handled by data tiles)."""
    for l in range(4):
        for si, b in enumerate(lane_blocks(l)):
            R = SLOT_R[si]
            qs = slice(b * P, (b + 1) * P)
            if R * P < cfg.S:
                if not (mask[qs, R * P:] <= -1e8).all():
                    return False
            lo = max(0, (R - 4)) * P
            if not (mask[qs, :lo] == 0).all():
                return False
    return True


def make_in_maps(x, freqs_cis, mask, wq, wk, wv, wo, cfg: Cfg):
    S, D, T, HD, DT = cfg.S, cfg.D, cfg.T, cfg.HD, cfg.DT
    NCH = 4 * cfg.TS
    NEH = cfg.NKV * HD // 512
    NDO = D // 512
    SCALE = np.float32(1.0) / np.float32(np.sqrt(np.float32(HD)))
    x = np.asarray(x, np.float32)
    fc = np.asarray(freqs_cis, np.float32)
    mask = np.asarray(mask, np.float32)
    wqt = np.asarray(wq, np.float32).T.astype(NPBF16)   # [D, NH*HD]
    wkt = np.asarray(wk, np.float32).T.astype(NPBF16)   # [D, KVW]
    wvt = np.asarray(wv, np.float32).T.astype(NPBF16)
    wot = np.asarray(wo, np.float32).T.astype(NPBF16)   # [NH*HD, D]

    wqp = pack_colgroups(wqt, groups_of3(cfg.NH), DT)
    wkp = pack_colgroups(wkt, groups_of3(cfg.NKV), DT)
    wvp = np.concatenate([
        np.ascontiguousarray(wvt[d * P:(d + 1) * P, eh * 512:(eh + 1) * 512])
        .reshape(-1)
        for eh in range(NEH) for d in range(DT)])
    wop = np.concatenate([
        np.ascontiguousarray(wot[e * P:(e + 1) * P, douth * 512:(douth + 1) * 512])
        .reshape(-1)
        for douth in range(NDO) for e in range(cfg.NH)])

    swapm = np.zeros((P, P), np.float32)
    for i in range(P // 2):
        swapm[2 * i, 2 * i + 1] = 1.0
        swapm[2 * i + 1, 2 * i] = 1.0
    swapm = swapm.astype(NPBF16)
    onesmat = np.ones((P, P), NPBF16)
    ident = np.concatenate([np.eye(P, dtype=np.float32)] * 2, axis=1).astype(NPBF16)

    in_maps = []
    for c in range(8):
        b, l = c // 4, c % 4
        blocks = lane_blocks(l)
        toks = np.concatenate([np.arange(bb * P, (bb + 1) * P) for bb in blocks])
        xt = np.ascontiguousarray(x[b][toks, :].T).astype(NPBF16)
        cost = np.repeat(fc[toks, :, 0].T, 2, axis=0).astype(np.float32)
        sint = np.repeat(fc[toks, :, 1].T, 2, axis=0).astype(np.float32)
        sint[0::2, :] *= -1.0
        # mask tile for chunk i: [128 q of the masked slot block, 128 k]
        maskq = np.zeros((P, NCH * P), np.float32)
        for i in range(NCH):
            si = 3 - i // 4                     # slot index masked by chunk i
            bb = blocks[si]
            maskq[:, i * P:(i + 1) * P] = \
                mask[bb * P:(bb + 1) * P, i * P:(i + 1) * P] / SCALE
        in_maps.append({
            "xt": xt, "wqp": wqp, "wkp": wkp, "wvp": wvp, "wop": wop,
            "cost": np.ascontiguousarray(cost),
            "sint": np.ascontiguousarray(sint),
            "maskq": np.ascontiguousarray(maskq).astype(NPBF16),
            "ident": ident, "swapm": swapm, "onesmat": onesmat,
        })
    return in_maps


_NC_CACHE = {}


def kernel_run(x, start_pos, freqs_cis, mask, wq, wk, wv, wo,
               cfg: Cfg = FULL, trace=False):
    mask_np = np.asarray(mask, np.float32)
    assert check_mask_structure(mask_np, cfg), \
        "mask incompatible with compiled causal structure"
    in_maps = make_in_maps(x, freqs_cis, mask, wq, wk, wv, wo, cfg)
    if cfg not in _NC_CACHE:
        _NC_CACHE[cfg] = build_nc(cfg)
    nc = _NC_CACHE[cfg]
    res = run_bass_kernel_spmd(nc, in_maps, core_ids=list(range(8)), trace=trace)
    full = np.zeros((2, cfg.S, cfg.D), np.float32)
    for c in range(8):
        b, l = c // 4, c % 4
        toks = np.concatenate(
            [np.arange(bb * P, (bb + 1) * P) for bb in lane_blocks(l)])
        full[b][toks, :] = res.results[c]["out"]
    return full, res


def kernel(x, start_pos=None, freqs_cis=None, mask=None, wq=None, wk=None,
           wv=None, wo=None):
    full, _ = kernel_run(x, start_pos, freqs_cis, mask, wq, wk, wv, wo)
    return full
